# revision 24
# baseline (speedup 1.0000x reference)
"""Distributed causal attention head for TRN2 (8 NeuronCores), v5.

Problem: B=4, S=4096, D=1024, H=64 fp32.
  q,k,v = x @ W{q,k,v}; scores = q k^T / sqrt(H); causal softmax; out = P v.

Design (collective-free, one SPMD-uniform NEFF, no barrier/AllGather/RS):
  - 4 batches x 2 cores per batch. Each core receives the FULL batch x^T
    (bf16, host-pretransposed) and projects Q^T for ALL 4096 queries
    itself; K/V only for the 2048 interleaved key rows it owns
    (128-chunk interleave keeps the causal work perfectly balanced).
  - SPMD uniformity: the host permutes x^T columns per core so the
    core's OWN key chunks sit at even 128-chunk positions. All extraction
    addresses are then identical across cores; causality differences are
    absorbed into per-core 0/1 mask DATA (queries are consistently in the
    permuted order; the host un-permutes the output).
  - v5 (this file): the whole schedule is re-pipelined around the PE
    p-state ramp and the Scalar exp stream. The Tensor engine only
    reaches 2.4GHz after ~3us of gap-free execution; any stall drops
    it back to 1.2GHz. The v4 layout shipped x as two 4MiB halves, so
    the first projection couldn't finish until ~16us and the PE
    crawled at 1.2GHz with multi-us DMA stalls until ~36us. Here x^T
    is host-packed into 8 block-major 1MiB blocks (512 query rows
    each, all 8 dc-chunks contiguous per partition -> 8KB
    descriptors), halves spread over three DMA queues so block k
    lands at ~11+3.1k us. Queue placement is load-bearing twice over:
    (1) a DMA-issue instruction occupies its engine until a hardware
    queue slot frees (~3 outstanding), so the scalar engine gets at
    most 3 issues or the exp stream is head-of-line blocked for 15+us;
    (2) the gpsimd SWDGE queue has ~4us extra startup, so it only
    carries late blocks + late-needed weights.
  - per-block emission: projection of block b, V chunks of block b,
    score pairs (t=b, ip<=b) and AV drains, so the PE saturates from
    ~block 1 on and holds full p-state. The exp stream (~1.1us per
    1024-wide score pair, 40us total) is the critical path: score
    matmuls + the qT/kT casts feeding it are tagged tc.high_priority.
    Score-pair matmuls reuse 2 PSUM banks, so pair j+2's matmul
    stalls the PE (resetting the p-state) if exp(j) hasn't drained; a
    work-debt pacer inserts ~1500 columns of projection/V/AV work
    between consecutive score pairs, which keeps the endgame in the
    fast equilibrium: PE at 2.4GHz delivering [score, AV, AV] per
    1.09us exp period with sub-100ns waits. (The slow equilibrium is
    real: bursting score pairs back-to-back makes the PE eat ~1us
    PSUM-recycle stalls, drop to 1.2GHz, and the whole kernel runs
    ~15% slower.)
  - V is augmented with a ones column so AV also emits the softmax
    denominator. Per-core partial (num^T | den) = [65, 4096] f32 goes
    straight to DRAM; the HOST adds the two partials of each pair,
    divides, and transposes. No on-device collective at all.
"""

import sys

sys.path.insert(0, "/opt/trn_rl_repo")

import numpy as np
import ml_dtypes

B, S, D, H = 4, 4096, 1024, 64
RPC = S // 2            # key rows owned per core
QB = 512                # query block width
NQB = S // QB           # 8 query blocks
NKC = RPC // 128        # 16 local key chunks
BF16 = ml_dtypes.bfloat16

_CACHE = {}


def _build():
    import concourse.bass as bass
    import concourse.mybir as mybir
    from concourse import bacc, tile
    from concourse.bass import ts

    f32 = mybir.dt.float32
    bf16 = mybir.dt.bfloat16
    Alu = mybir.AluOpType
    Act = mybir.ActivationFunctionType

    nc = bacc.Bacc(None, target_bir_lowering=False)

    # x^T block-major: [p, blk, dc, col]; one block = 512 query rows,
    # 8KB contiguous per partition
    xt_ext = nc.declare_dram_parameter("xt", [128, NQB, 8, QB], bf16, isOutput=False)
    # weights pre-shuffled on host: partition p holds all 8 dc-chunks
    wqk_ext = nc.declare_dram_parameter("wqk", [128, 8 * 128], bf16, isOutput=False)
    wv_ext = nc.declare_dram_parameter("wv", [128, 8 * H], bf16, isOutput=False)
    mask_ext = nc.declare_dram_parameter("mask", [128, 1024], bf16, isOutput=False)
    out_ext = nc.declare_dram_parameter("out", [H + 1, S], f32, isOutput=True)

    pairs = [(t, ip) for t in range(NQB) for ip in range(t + 1)]  # 36 chunk-pairs

    with tile.TileContext(nc) as tc:
        with tc.tile_pool(name="persist", bufs=1) as persist:
            # --- persistent SBUF tensors ---
            wqk_sb = persist.tile([128, 8, 128], bf16, tag="wqk")
            wv_sb = persist.tile([128, 8, H], bf16, tag="wv")
            mask_sb = persist.tile([128, 1024], bf16, tag="mask")
            qT = persist.tile([64, S], bf16, tag="qT")
            kT = persist.tile([64, RPC], bf16, tag="kT")
            v_all = persist.tile([128, NKC, H + 1], bf16, tag="v_all")
            p_sb = persist.tile([128, 36, 1024], bf16, tag="p")
            xt_sb = persist.tile([128, NQB, 8, QB], bf16, tag="xt")

            nc.vector.memset(v_all[:, :, H], 1.0)

            # DMA plan. Three queues: sync + scalar are hardware queues
            # (~165GB/s each, live at ~7.2us); gpsimd is the SWDGE queue
            # with ~4us extra startup. The DMA-issue instruction occupies
            # its engine until a queue slot frees (~3 outstanding), so the
            # scalar engine gets at most 3 issues -- any more would
            # head-of-line-block the exp stream, which costs ~1.1us per
            # deferred exp. Layout makes block b complete at ~10.5+3.2b us:
            #   scalar: b0.lo, b1.lo, b2.lo        (done 10.3/13.4/16.5us)
            #   sync:   wqk, b0.hi, b1.hi, b3..b7 .lo  (8.7/11.8/14.9/18..30.4)
            #   gpsimd: wv, mask, b2..b7 .hi       (~12/~13/16.4..31.9us)
            nc.scalar.dma_start(out=xt_sb[:, 0, 0:4, :], in_=xt_ext[:, 0, 0:4, :])
            nc.scalar.dma_start(out=xt_sb[:, 1, 0:4, :], in_=xt_ext[:, 1, 0:4, :])
            nc.scalar.dma_start(out=xt_sb[:, 2, 0:4, :], in_=xt_ext[:, 2, 0:4, :])
            nc.sync.dma_start(out=wqk_sb[:], in_=wqk_ext[:])
            nc.sync.dma_start(out=xt_sb[:, 0, 4:8, :], in_=xt_ext[:, 0, 4:8, :])
            nc.sync.dma_start(out=xt_sb[:, 1, 4:8, :], in_=xt_ext[:, 1, 4:8, :])
            nc.gpsimd.dma_start(out=wv_sb[:], in_=wv_ext[:])
            nc.gpsimd.dma_start(out=mask_sb[:], in_=mask_ext[:])
            nc.gpsimd.dma_start(out=xt_sb[:, 2, 4:8, :], in_=xt_ext[:, 2, 4:8, :])
            for blk in range(3, NQB):
                nc.sync.dma_start(
                    out=xt_sb[:, blk, 0:4, :], in_=xt_ext[:, blk, 0:4, :]
                )
                nc.gpsimd.dma_start(
                    out=xt_sb[:, blk, 4:8, :], in_=xt_ext[:, blk, 4:8, :]
                )

            with (
                tc.tile_pool(name="qk", bufs=1, space="PSUM") as qk_pool,
                tc.tile_pool(name="pv", bufs=1, space="PSUM") as pv_pool,
                tc.tile_pool(name="st", bufs=2, space="PSUM") as st_pool,
                tc.tile_pool(name="av", bufs=2, space="PSUM") as av_pool,
                tc.tile_pool(name="o", bufs=2) as o_pool,
            ):
                state = {"st": 0, "debt": 0, "av": 0, "v": -1}
                av_tiles = {}

                def emit_st_pair():
                    j = state["st"]
                    t, ip = pairs[j]
                    st2 = st_pool.tile([128, 1024], f32, tag="st", name=f"st{j}")
                    # the exp stream is the kernel's critical path: its score
                    # matmuls must win scheduler ties against V/AV fill work
                    with tc.high_priority():
                        nc.tensor.matmul(
                            st2[:, 0:512],
                            lhsT=kT[:, 256 * ip : 256 * ip + 128],
                            rhs=qT[:, ts(t, QB)],
                            start=True,
                            stop=True,
                            skip_group_check=True,
                        )
                        nc.tensor.matmul(
                            st2[:, 512:1024],
                            lhsT=kT[:, 256 * ip + 128 : 256 * ip + 256],
                            rhs=qT[:, ts(t, QB)],
                            start=True,
                            stop=True,
                            skip_group_check=True,
                        )
                    nc.scalar.activation(p_sb[:, j, :], st2[:], Act.Exp, scale=0.125)
                    if ip == t:  # diagonal pair: multiplicative causal mask
                        nc.vector.tensor_tensor(
                            p_sb[:, j, :], p_sb[:, j, :], mask_sb[:], Alu.mult
                        )
                    state["st"] += 1
                    # ~1500 cols of non-score work must follow before the
                    # next pair so the PE never waits on the exp stream
                    state["debt"] = 1500

                def emit_av_pair():
                    j = state["av"]
                    t, ip = pairs[j]
                    if ip == 0:
                        av_tiles[t] = av_pool.tile(
                            [H + 1, QB], f32, tag="av", name=f"av{t}"
                        )
                    av = av_tiles[t]
                    nc.tensor.matmul(
                        av[:],
                        lhsT=v_all[:, 2 * ip, :],
                        rhs=p_sb[:, j, 0:512],
                        start=(ip == 0),
                        stop=False,
                        skip_group_check=True,
                    )
                    nc.tensor.matmul(
                        av[:],
                        lhsT=v_all[:, 2 * ip + 1, :],
                        rhs=p_sb[:, j, 512:1024],
                        start=False,
                        stop=(ip == t),
                        skip_group_check=True,
                    )
                    if ip == t:
                        o_sb = o_pool.tile([H + 1, QB], f32, tag="o", name=f"o{t}")
                        nc.vector.tensor_copy(o_sb[:], av[:])
                        nc.sync.dma_start(out=out_ext[:, ts(t, QB)], in_=o_sb[:])
                    state["av"] += 1
                    state["debt"] -= 1024

                def av_ready():
                    return (
                        state["av"] < state["st"] - 2
                        and 2 * pairs[state["av"]][1] + 1 <= state["v"]
                    )

                def maybe_st_pair(t_max, force=False):
                    if state["st"] < 36 and pairs[state["st"]][0] <= t_max:
                        if force or state["debt"] <= 0:
                            emit_st_pair()
                            return True
                    return False

                def proj_qk(blk):
                    qkp = qk_pool.tile([128, QB], f32, tag="qk", name=f"qk{blk}")
                    # projection + casts feed the score pairs feeding the exp
                    # stream -- all on the critical path
                    with tc.high_priority():
                        for dc in range(8):
                            nc.tensor.matmul(
                                qkp[:],
                                lhsT=wqk_sb[:, dc, :],
                                rhs=xt_sb[:, blk, dc, :],
                                start=(dc == 0),
                                stop=(dc == 7),
                                skip_group_check=True,
                            )
                        # qT cast first: block b's first score pairs need
                        # qT(b) with OLD kT chunks; kT(b) is only needed by
                        # the diagonal pair, last of the block
                        nc.vector.tensor_copy(qT[:, ts(blk, QB)], qkp[0:64, :])
                        nc.vector.tensor_copy(
                            kT[:, 256 * blk : 256 * blk + 128], qkp[64:128, 0:128]
                        )
                        nc.vector.tensor_copy(
                            kT[:, 256 * blk + 128 : 256 * blk + 256],
                            qkp[64:128, 256:384],
                        )
                    state["debt"] -= 4096

                def emit_v_chunk(i):
                    blk, ii = i // 2, i % 2
                    vps = pv_pool.tile([128, H], f32, tag="v", name=f"v{i}")
                    for dc in range(8):
                        nc.tensor.matmul(
                            vps[:],
                            lhsT=xt_sb[:, blk, dc, 256 * ii : 256 * ii + 128],
                            rhs=wv_sb[:, dc, :],
                            start=(dc == 0),
                            stop=(dc == 7),
                        )
                    nc.vector.tensor_copy(v_all[:, i, 0:H], vps[:])
                    state["v"] = i
                    state["debt"] -= 512

                # --- per-block pipeline ---
                for blk in range(NQB):
                    proj_qk(blk)
                    for ii in range(2):
                        maybe_st_pair(blk)
                        emit_v_chunk(2 * blk + ii)
                    # emit the ready score pairs, spaced by AV drains
                    while True:
                        progress = maybe_st_pair(blk)
                        if av_ready():
                            emit_av_pair()
                            progress = True
                        if not progress:
                            break

                # --- tail: remaining score pairs + AV drains ---
                while state["st"] < 36:
                    if not maybe_st_pair(NQB - 1):
                        if av_ready():
                            emit_av_pair()
                        else:
                            maybe_st_pair(NQB - 1, force=True)
                while state["av"] < 36:
                    emit_av_pair()

    nc.finalize()
    return nc


def _make_mask2(g: int) -> np.ndarray:
    """[128, 1024] multiplicative mask for the diagonal chunk pair of any
    query block t (t-independent thanks to the per-core permutation).

    Query columns are in permuted order: position pc in the block maps to
    global query chunk offsets delta = [g, 1-g, 2+g, 3-g] (relative to 4t).
    Left half masks own key chunk at global offset g; right half offset 2+g.
    """
    m = np.zeros((128, 1024), dtype=np.float32)
    delta = [g, 1 - g, 2 + g, 3 - g]
    kk = np.arange(128)[:, None]
    qq = np.arange(128)[None, :]
    for half, keyoff in ((0, g), (1, 2 + g)):
        for pc in range(4):
            keep = (128 * (delta[pc] - keyoff) + qq) >= kk
            m[:, half * 512 + pc * 128 : half * 512 + (pc + 1) * 128] = keep
    return m.astype(BF16)


def _swap_pairs(a: np.ndarray) -> np.ndarray:
    """Swap adjacent 128-column chunks (self-inverse permutation)."""
    n = a.shape[-1]
    return np.ascontiguousarray(
        a.reshape(a.shape[:-1] + (n // 256, 2, 128))[..., ::-1, :].reshape(a.shape)
    )


def _pack_blocks(xt: np.ndarray) -> np.ndarray:
    """[D, S] x^T -> block-major [128, blk, dc, col]."""
    return np.ascontiguousarray(
        xt.reshape(8, 128, NQB, QB).transpose(1, 2, 0, 3)
    )


def _shard_inputs(input, Wq, Wk, Wv):
    wqk = np.concatenate([Wq, Wk], axis=1).astype(BF16)       # [1024, 128]
    wv = np.asarray(Wv).astype(BF16)                          # [1024, 64]
    # partition-major reshuffle so the SBUF load uses 1-2KB descriptors:
    # partition p holds [dc, col] for all 8 dc chunks
    wqk_r = np.ascontiguousarray(
        wqk.reshape(8, 128, 128).transpose(1, 0, 2).reshape(128, 8 * 128)
    )
    wv_r = np.ascontiguousarray(
        wv.reshape(8, 128, H).transpose(1, 0, 2).reshape(128, 8 * H)
    )
    masks = [_make_mask2(0), _make_mask2(1)]
    in_maps = []
    for b in range(B):
        xt = np.ascontiguousarray(np.asarray(input)[b].T).astype(BF16)
        xb = [_pack_blocks(xt), _pack_blocks(_swap_pairs(xt))]
        for g in range(2):
            in_maps.append(
                {
                    "xt": xb[g],
                    "wqk": wqk_r,
                    "wv": wv_r,
                    "mask": masks[g],
                }
            )
    return in_maps


def _unshard(results):
    out = np.empty((B, S, H), dtype=np.float32)
    for b in range(B):
        r0 = results[2 * b]["out"]                      # [65, S] natural order
        r1 = _swap_pairs(results[2 * b + 1]["out"])     # un-permute g=1
        m = r0 + r1
        out[b] = (m[:H] / m[H : H + 1]).T
    return out


def _run(inputs, trace=False):
    from concourse.bass_utils import run_bass_kernel_spmd

    if "nc" not in _CACHE:
        _CACHE["nc"] = _build()
    nc = _CACHE["nc"]
    in_maps = _shard_inputs(**inputs)
    res = run_bass_kernel_spmd(nc, in_maps, core_ids=list(range(8)), trace=trace)
    out = _unshard(res.results)
    return out, res


def kernel(**inputs) -> np.ndarray:
    out, _ = _run(inputs, trace=False)
    return out


# revision 26
# speedup vs baseline: 1.0063x; 1.0063x over previous
"""Distributed causal attention head for TRN2 (8 NeuronCores), v5.

Problem: B=4, S=4096, D=1024, H=64 fp32.
  q,k,v = x @ W{q,k,v}; scores = q k^T / sqrt(H); causal softmax; out = P v.

Design (collective-free, one SPMD-uniform NEFF, no barrier/AllGather/RS):
  - 4 batches x 2 cores per batch. Each core receives the FULL batch x^T
    (bf16, host-pretransposed) and projects Q^T for ALL 4096 queries
    itself; K/V only for the 2048 interleaved key rows it owns
    (128-chunk interleave keeps the causal work perfectly balanced).
  - SPMD uniformity: the host permutes x^T columns per core so the
    core's OWN key chunks sit at even 128-chunk positions. All extraction
    addresses are then identical across cores; causality differences are
    absorbed into per-core 0/1 mask DATA (queries are consistently in the
    permuted order; the host un-permutes the output).
  - v5 (this file): the whole schedule is re-pipelined around the PE
    p-state ramp and the Scalar exp stream. The Tensor engine only
    reaches 2.4GHz after ~3us of gap-free execution; any stall drops
    it back to 1.2GHz. The v4 layout shipped x as two 4MiB halves, so
    the first projection couldn't finish until ~16us and the PE
    crawled at 1.2GHz with multi-us DMA stalls until ~36us. Here x^T
    is host-packed into 8 block-major 1MiB blocks (512 query rows
    each, all 8 dc-chunks contiguous per partition -> 8KB
    descriptors), halves spread over three DMA queues so block k
    lands at ~11+3.1k us. Queue placement is load-bearing twice over:
    (1) a DMA-issue instruction occupies its engine until a hardware
    queue slot frees (~3 outstanding), so the scalar engine gets at
    most 3 issues or the exp stream is head-of-line blocked for 15+us;
    (2) the gpsimd SWDGE queue has ~4us extra startup, so it only
    carries late blocks + late-needed weights.
  - per-block emission: projection of block b, V chunks of block b,
    score pairs (t=b, ip<=b) and AV drains, so the PE saturates from
    ~block 1 on and holds full p-state. The exp stream (~1.1us per
    1024-wide score pair, 40us total) is the critical path: score
    matmuls + the qT/kT casts feeding it are tagged tc.high_priority.
    Score-pair matmuls reuse 2 PSUM banks, so pair j+2's matmul
    stalls the PE (resetting the p-state) if exp(j) hasn't drained; a
    work-debt pacer inserts ~1500 columns of projection/V/AV work
    between consecutive score pairs, which keeps the endgame in the
    fast equilibrium: PE at 2.4GHz delivering [score, AV, AV] per
    1.09us exp period with sub-100ns waits. (The slow equilibrium is
    real: bursting score pairs back-to-back makes the PE eat ~1us
    PSUM-recycle stalls, drop to 1.2GHz, and the whole kernel runs
    ~15% slower.)
  - V is augmented with a ones column so AV also emits the softmax
    denominator. Per-core partial (num^T | den) = [65, 4096] f32 goes
    straight to DRAM; the HOST adds the two partials of each pair,
    divides, and transposes. No on-device collective at all.
"""

import sys

sys.path.insert(0, "/opt/trn_rl_repo")

import numpy as np
import ml_dtypes

B, S, D, H = 4, 4096, 1024, 64
RPC = S // 2            # key rows owned per core
QB = 512                # query block width
NQB = S // QB           # 8 query blocks
NKC = RPC // 128        # 16 local key chunks
BF16 = ml_dtypes.bfloat16

_CACHE = {}


def _build():
    import concourse.bass as bass
    import concourse.mybir as mybir
    from concourse import bacc, tile
    from concourse.bass import ts

    f32 = mybir.dt.float32
    bf16 = mybir.dt.bfloat16
    Alu = mybir.AluOpType
    Act = mybir.ActivationFunctionType

    nc = bacc.Bacc(None, target_bir_lowering=False)

    # x^T block-major: [p, blk, dc, col]; one block = 512 query rows,
    # 8KB contiguous per partition
    xt_ext = nc.declare_dram_parameter("xt", [128, NQB, 8, QB], bf16, isOutput=False)
    # weights pre-shuffled on host: partition p holds all 8 dc-chunks
    wqk_ext = nc.declare_dram_parameter("wqk", [128, 8 * 128], bf16, isOutput=False)
    wv_ext = nc.declare_dram_parameter("wv", [128, 8 * H], bf16, isOutput=False)
    mask_ext = nc.declare_dram_parameter("mask", [128, 1024], bf16, isOutput=False)
    out_ext = nc.declare_dram_parameter("out", [H + 1, S], f32, isOutput=True)

    pairs = [(t, ip) for t in range(NQB) for ip in range(t + 1)]  # 36 chunk-pairs

    with tile.TileContext(nc) as tc:
        with tc.tile_pool(name="persist", bufs=1) as persist:
            # --- persistent SBUF tensors ---
            wqk_sb = persist.tile([128, 8, 128], bf16, tag="wqk")
            wv_sb = persist.tile([128, 8, H], bf16, tag="wv")
            mask_sb = persist.tile([128, 1024], bf16, tag="mask")
            qT = persist.tile([64, S], bf16, tag="qT")
            kT = persist.tile([64, RPC], bf16, tag="kT")
            v_all = persist.tile([128, NKC, H + 1], bf16, tag="v_all")
            p_sb = persist.tile([128, 36, 1024], bf16, tag="p")
            xt_sb = persist.tile([128, NQB, 8, QB], bf16, tag="xt")

            nc.vector.memset(v_all[:, :, H], 1.0)

            # DMA plan. Three queues: sync + scalar are hardware queues
            # (~165GB/s each, live at ~7.2us); gpsimd is the SWDGE queue
            # with ~4us extra startup. The DMA-issue instruction occupies
            # its engine until a queue slot frees (~3 outstanding), so the
            # scalar engine gets at most 3 issues -- any more would
            # head-of-line-block the exp stream, which costs ~1.1us per
            # deferred exp. Layout makes block b complete at ~10.5+3.2b us:
            #   scalar: b0.lo, b1.lo, b2.lo        (done 10.3/13.4/16.5us)
            #   sync:   wqk, b0.hi, b1.hi, b3..b7 .lo  (8.7/11.8/14.9/18..30.4)
            #   gpsimd: wv, mask, b2..b7 .hi       (~12/~13/16.4..31.9us)
            nc.scalar.dma_start(out=xt_sb[:, 0, 0:4, :], in_=xt_ext[:, 0, 0:4, :])
            nc.scalar.dma_start(out=xt_sb[:, 1, 0:4, :], in_=xt_ext[:, 1, 0:4, :])
            nc.scalar.dma_start(out=xt_sb[:, 2, 0:4, :], in_=xt_ext[:, 2, 0:4, :])
            nc.sync.dma_start(out=wqk_sb[:], in_=wqk_ext[:])
            nc.sync.dma_start(out=xt_sb[:, 0, 4:8, :], in_=xt_ext[:, 0, 4:8, :])
            nc.sync.dma_start(out=xt_sb[:, 1, 4:8, :], in_=xt_ext[:, 1, 4:8, :])
            nc.gpsimd.dma_start(out=wv_sb[:], in_=wv_ext[:])
            nc.gpsimd.dma_start(out=mask_sb[:], in_=mask_ext[:])
            nc.gpsimd.dma_start(out=xt_sb[:, 2, 4:8, :], in_=xt_ext[:, 2, 4:8, :])
            for blk in range(3, NQB):
                nc.sync.dma_start(
                    out=xt_sb[:, blk, 0:4, :], in_=xt_ext[:, blk, 0:4, :]
                )
                nc.gpsimd.dma_start(
                    out=xt_sb[:, blk, 4:8, :], in_=xt_ext[:, blk, 4:8, :]
                )

            with (
                tc.tile_pool(name="qk", bufs=1, space="PSUM") as qk_pool,
                tc.tile_pool(name="pv", bufs=1, space="PSUM") as pv_pool,
                tc.tile_pool(name="st", bufs=2, space="PSUM") as st_pool,
                tc.tile_pool(name="av", bufs=2, space="PSUM") as av_pool,
                tc.tile_pool(name="o", bufs=2) as o_pool,
            ):
                state = {"st": 0, "debt": 0, "av": 0, "v": -1}
                av_tiles = {}

                def emit_st_pair():
                    j = state["st"]
                    t, ip = pairs[j]
                    st2 = st_pool.tile([128, 1024], f32, tag="st", name=f"st{j}")
                    # the exp stream is the kernel's critical path: its score
                    # matmuls must win scheduler ties against V/AV fill work
                    with tc.high_priority():
                        nc.tensor.matmul(
                            st2[:, 0:512],
                            lhsT=kT[:, 256 * ip : 256 * ip + 128],
                            rhs=qT[:, ts(t, QB)],
                            start=True,
                            stop=True,
                            skip_group_check=True,
                        )
                        nc.tensor.matmul(
                            st2[:, 512:1024],
                            lhsT=kT[:, 256 * ip + 128 : 256 * ip + 256],
                            rhs=qT[:, ts(t, QB)],
                            start=True,
                            stop=True,
                            skip_group_check=True,
                        )
                    nc.scalar.activation(p_sb[:, j, :], st2[:], Act.Exp, scale=0.125)
                    if ip == t:  # diagonal pair: multiplicative causal mask
                        nc.vector.tensor_tensor(
                            p_sb[:, j, :], p_sb[:, j, :], mask_sb[:], Alu.mult
                        )
                    state["st"] += 1
                    # non-score work must follow before the next pair so the
                    # PE never waits on the exp stream; alternating 1024/1536
                    # yields [st,av] / [st,av,av] alternation = 1.07us/pair
                    # average, matching the 1.086us exp cadence (a fixed
                    # [st,av,av] = 1.28us/pair starves exp ~0.2us per pair)
                    state["debt"] = 1024 if (j & 1) else 1536

                def emit_av_pair():
                    j = state["av"]
                    t, ip = pairs[j]
                    if ip == 0:
                        av_tiles[t] = av_pool.tile(
                            [H + 1, QB], f32, tag="av", name=f"av{t}"
                        )
                    av = av_tiles[t]
                    nc.tensor.matmul(
                        av[:],
                        lhsT=v_all[:, 2 * ip, :],
                        rhs=p_sb[:, j, 0:512],
                        start=(ip == 0),
                        stop=False,
                        skip_group_check=True,
                    )
                    nc.tensor.matmul(
                        av[:],
                        lhsT=v_all[:, 2 * ip + 1, :],
                        rhs=p_sb[:, j, 512:1024],
                        start=False,
                        stop=(ip == t),
                        skip_group_check=True,
                    )
                    if ip == t:
                        o_sb = o_pool.tile([H + 1, QB], f32, tag="o", name=f"o{t}")
                        nc.vector.tensor_copy(o_sb[:], av[:])
                        nc.sync.dma_start(out=out_ext[:, ts(t, QB)], in_=o_sb[:])
                    state["av"] += 1
                    state["debt"] -= 1024

                def av_ready():
                    return (
                        state["av"] < state["st"] - 2
                        and 2 * pairs[state["av"]][1] + 1 <= state["v"]
                    )

                def maybe_st_pair(t_max, force=False):
                    if state["st"] < 36 and pairs[state["st"]][0] <= t_max:
                        if force or state["debt"] <= 0:
                            emit_st_pair()
                            return True
                    return False

                def proj_qk(blk):
                    qkp = qk_pool.tile([128, QB], f32, tag="qk", name=f"qk{blk}")
                    # projection + casts feed the score pairs feeding the exp
                    # stream -- all on the critical path
                    with tc.high_priority():
                        for dc in range(8):
                            nc.tensor.matmul(
                                qkp[:],
                                lhsT=wqk_sb[:, dc, :],
                                rhs=xt_sb[:, blk, dc, :],
                                start=(dc == 0),
                                stop=(dc == 7),
                                skip_group_check=True,
                            )
                        # qT cast first: block b's first score pairs need
                        # qT(b) with OLD kT chunks; kT(b) is only needed by
                        # the diagonal pair, last of the block
                        nc.vector.tensor_copy(qT[:, ts(blk, QB)], qkp[0:64, :])
                        nc.vector.tensor_copy(
                            kT[:, 256 * blk : 256 * blk + 128], qkp[64:128, 0:128]
                        )
                        nc.vector.tensor_copy(
                            kT[:, 256 * blk + 128 : 256 * blk + 256],
                            qkp[64:128, 256:384],
                        )
                    state["debt"] -= 4096

                def emit_v_chunk(i):
                    blk, ii = i // 2, i % 2
                    vps = pv_pool.tile([128, H], f32, tag="v", name=f"v{i}")
                    for dc in range(8):
                        nc.tensor.matmul(
                            vps[:],
                            lhsT=xt_sb[:, blk, dc, 256 * ii : 256 * ii + 128],
                            rhs=wv_sb[:, dc, :],
                            start=(dc == 0),
                            stop=(dc == 7),
                        )
                    nc.vector.tensor_copy(v_all[:, i, 0:H], vps[:])
                    state["v"] = i
                    state["debt"] -= 512

                # --- per-block pipeline ---
                for blk in range(NQB):
                    proj_qk(blk)
                    for ii in range(2):
                        maybe_st_pair(blk)
                        emit_v_chunk(2 * blk + ii)
                    # emit the ready score pairs, spaced by AV drains.
                    # Through block 3 the exp stream is supply-starved (it
                    # drains pairs faster than DMA+proj can deliver), so
                    # emit every ready pair immediately -- deferring one
                    # behind the NEXT block's projection costs exp ~3us.
                    # From block 4 on exp is saturated and bursts would
                    # stall the PE on PSUM recycling, so debt-pace.
                    while True:
                        progress = maybe_st_pair(blk, force=(blk <= 3))
                        if av_ready():
                            emit_av_pair()
                            progress = True
                        if not progress:
                            break

                # --- tail: remaining score pairs + AV drains ---
                while state["st"] < 36:
                    if not maybe_st_pair(NQB - 1):
                        if av_ready():
                            emit_av_pair()
                        else:
                            maybe_st_pair(NQB - 1, force=True)
                while state["av"] < 36:
                    emit_av_pair()

    nc.finalize()
    return nc


def _make_mask2(g: int) -> np.ndarray:
    """[128, 1024] multiplicative mask for the diagonal chunk pair of any
    query block t (t-independent thanks to the per-core permutation).

    Query columns are in permuted order: position pc in the block maps to
    global query chunk offsets delta = [g, 1-g, 2+g, 3-g] (relative to 4t).
    Left half masks own key chunk at global offset g; right half offset 2+g.
    """
    m = np.zeros((128, 1024), dtype=np.float32)
    delta = [g, 1 - g, 2 + g, 3 - g]
    kk = np.arange(128)[:, None]
    qq = np.arange(128)[None, :]
    for half, keyoff in ((0, g), (1, 2 + g)):
        for pc in range(4):
            keep = (128 * (delta[pc] - keyoff) + qq) >= kk
            m[:, half * 512 + pc * 128 : half * 512 + (pc + 1) * 128] = keep
    return m.astype(BF16)


def _swap_pairs(a: np.ndarray) -> np.ndarray:
    """Swap adjacent 128-column chunks (self-inverse permutation)."""
    n = a.shape[-1]
    return np.ascontiguousarray(
        a.reshape(a.shape[:-1] + (n // 256, 2, 128))[..., ::-1, :].reshape(a.shape)
    )


def _pack_blocks(xt: np.ndarray) -> np.ndarray:
    """[D, S] x^T -> block-major [128, blk, dc, col]."""
    return np.ascontiguousarray(
        xt.reshape(8, 128, NQB, QB).transpose(1, 2, 0, 3)
    )


def _shard_inputs(input, Wq, Wk, Wv):
    wqk = np.concatenate([Wq, Wk], axis=1).astype(BF16)       # [1024, 128]
    wv = np.asarray(Wv).astype(BF16)                          # [1024, 64]
    # partition-major reshuffle so the SBUF load uses 1-2KB descriptors:
    # partition p holds [dc, col] for all 8 dc chunks
    wqk_r = np.ascontiguousarray(
        wqk.reshape(8, 128, 128).transpose(1, 0, 2).reshape(128, 8 * 128)
    )
    wv_r = np.ascontiguousarray(
        wv.reshape(8, 128, H).transpose(1, 0, 2).reshape(128, 8 * H)
    )
    masks = [_make_mask2(0), _make_mask2(1)]
    in_maps = []
    for b in range(B):
        xt = np.ascontiguousarray(np.asarray(input)[b].T).astype(BF16)
        xb = [_pack_blocks(xt), _pack_blocks(_swap_pairs(xt))]
        for g in range(2):
            in_maps.append(
                {
                    "xt": xb[g],
                    "wqk": wqk_r,
                    "wv": wv_r,
                    "mask": masks[g],
                }
            )
    return in_maps


def _unshard(results):
    out = np.empty((B, S, H), dtype=np.float32)
    for b in range(B):
        r0 = results[2 * b]["out"]                      # [65, S] natural order
        r1 = _swap_pairs(results[2 * b + 1]["out"])     # un-permute g=1
        m = r0 + r1
        out[b] = (m[:H] / m[H : H + 1]).T
    return out


def _run(inputs, trace=False):
    from concourse.bass_utils import run_bass_kernel_spmd

    if "nc" not in _CACHE:
        _CACHE["nc"] = _build()
    nc = _CACHE["nc"]
    in_maps = _shard_inputs(**inputs)
    res = run_bass_kernel_spmd(nc, in_maps, core_ids=list(range(8)), trace=trace)
    out = _unshard(res.results)
    return out, res


def kernel(**inputs) -> np.ndarray:
    out, _ = _run(inputs, trace=False)
    return out
